# revision 37
# baseline (speedup 1.0000x reference)
"""Batch whitening (Cholesky) kernel for Trainium2, 8 NeuronCores.

Computes, for X [32768, 1024] (matching the reference nn_BWCholeskyBlock):
    mean = X.mean(0); xc = X - mean; cov = xc.T @ xc / N
    L = chol(cov + eps I);  Y = (L^-1 xc^T).T + beta

Strategy (data-parallel over batch, 8 cores):
  Phase 1 (device): per-core partial gram  G_i = X_i^T X_i  in fp8-e4m3
     with DoubleRow matmuls (2 k-tiles contracted per instruction = 2x PE
     throughput; only the 20 lower-triangle [128,256] tiles computed).
     Tiles are packed into one SBUF buffer and shipped with 2 large DMAs
     (SW-DGE descriptor reclaim at drain costs ~130ns per DMA issued, so
     few/large transfers beat many/small ones).
  Host: reduce partials (the all-reduce of the sharding hint), mirror the
     triangle; mean from a host column sum; cov; Cholesky + triangular
     inverse of the small [F,F] factor (replicated);  W = L^-1.
  Phase 2 (device): with  WT = W.T = I + E  (E upper triangular, small),
     Y = X @ WT + b = (X + b) + X @ E.   X+b ships as bf16 (the exact
     identity part), E ships as fp8 scaled by 64, X^T ships as fp8 for
     the stationary operand.  PE does only the 6 DoubleRow E-matmuls per
     row-tile; one DVE scalar_tensor_tensor per half fuses the 1/64
     rescale with the identity add; outputs are written per row-tile PAIR
     ([128,2,1024] = 1MB per DMA) alternating across two engine queues.
"""
import sys

sys.path.insert(0, "/opt/trn_rl_repo")

import numpy as np
import ml_dtypes

import concourse.bass as bass
import concourse.mybir as mybir
import concourse.tile as tile
from concourse import bacc
from concourse.bass_utils import run_bass_kernel_spmd

EPS = 1e-5
N_CORES = 8
N_TOTAL = 32768
F = 1024
NC_ROWS = N_TOTAL // N_CORES  # 4096 rows per core
NT = NC_ROWS // 128           # 32 row-tiles per core
NPAIR = NT // 2               # 16 DoubleRow pairs per core
P = 128
FH = F // 2                   # 512
FQ = F // 4                   # 256
KB = F // P                   # 8 column blocks of 128

F32 = mybir.dt.float32
BF16 = mybir.dt.bfloat16
F8 = mybir.dt.float8e4
DR = mybir.MatmulPerfMode.DoubleRow
ALU = mybir.AluOpType

NP_F8 = ml_dtypes.float8_e4m3
NP_BF16 = ml_dtypes.bfloat16

E_SCALE = 64.0  # E entries ~5e-3 sit in fp8 subnormal range; prescale

# gram tiles (mf, nq): rows mf*128..+128, cols nq*256..+256; the 20 tiles
# covering the diagonal/lower triangle, emitted grouped by stationary mf so
# consecutive matmuls share the loaded weights. Pass A = 16 tiles (8 PSUM
# banks, 2 half-bank accumulators each), pass B = 4 tiles.
PASS_A = [
    (0, 0), (1, 0), (2, 0), (2, 1), (3, 0), (3, 1), (4, 0), (4, 1),
    (4, 2), (5, 0), (5, 1), (5, 2), (6, 0), (6, 1), (7, 0), (7, 1),
]
PASS_B = [(6, 2), (6, 3), (7, 2), (7, 3)]
# PSUM->SBUF copy order: pass A pair-major (j, h), then pass B
COPY_ORDER = [PASS_A[j + h * 8] for j in range(8) for h in range(2)] + [
    PASS_B[j + h * 2] for j in range(2) for h in range(2)
]


def build_phase1() -> bass.Bass:
    """Per-core: lower-triangle gram tiles of X^T X (fp8 DoubleRow),
    packed as [128, 20, 256] in COPY_ORDER."""
    nc = bacc.Bacc(None, target_bir_lowering=False, debug=False)

    x_in = nc.dram_tensor("x", [NC_ROWS, F], F8, kind="ExternalInput")
    gram_out = nc.dram_tensor("gram", [P, 20, FQ], F32, kind="ExternalOutput")

    x_r = x_in.rearrange("(t p) f -> p t f", p=P)  # [128, 32, 1024]

    with tile.TileContext(nc) as tc:
        with (
            tc.tile_pool(name="xres", bufs=1) as xres,
            tc.tile_pool(name="gout", bufs=1) as gout,
            tc.tile_pool(name="psum", bufs=8, space="PSUM") as psum,
        ):
            # all of X SBUF-resident in fp8 (32 KiB per partition); few
            # large DMAs (first two small so the PE starts early), split
            # across two queues
            x8 = xres.tile([P, NT, F], F8)
            nc.sync.dma_start(out=x8[:, 0:2, :], in_=x_r[:, 0:2, :])
            nc.gpsimd.dma_start(out=x8[:, 2:4, :], in_=x_r[:, 2:4, :])
            nc.scalar.dma_start(out=x8[:, 4:8, :], in_=x_r[:, 4:8, :])
            nc.sync.dma_start(out=x8[:, 8:16, :], in_=x_r[:, 8:16, :])
            nc.gpsimd.dma_start(out=x8[:, 16:24, :], in_=x_r[:, 16:24, :])
            nc.scalar.dma_start(out=x8[:, 24:32, :], in_=x_r[:, 24:32, :])

            gpack = gout.tile([P, 20, FQ], F32)

            def pair(r):
                return x8[:, 2 * r : 2 * r + 2, :]

            # pass A: 16 gram tiles; two [128,256] fp32 accumulators share
            # one PSUM bank (bank b holds tiles idx b and b+8). start=True
            # zeroes the whole 2KB bank, so only idx<8 carries it; idx>=8
            # first matmuls land on the already-zeroed half.
            psA = [
                psum.tile([P, 2, FQ], F32, tag="g", name=f"gA_{i}")
                for i in range(8)
            ]
            for r in range(NPAIR):
                for i, (mf, nq) in enumerate(PASS_A):
                    nc.tensor.matmul(
                        psA[i % 8][:, i // 8, :],
                        pair(r)[:, :, mf * P : (mf + 1) * P],
                        pair(r)[:, :, nq * FQ : (nq + 1) * FQ],
                        start=(r == 0 and i < 8),
                        stop=(r == NPAIR - 1),
                        perf_mode=DR,
                    )

            slot = [0]

            def copy_out(src):
                s = slot[0]
                slot[0] = s + 1
                # GpSimd cannot read PSUM; alternate the PSUM-capable engines
                if s % 2 == 0:
                    nc.scalar.copy(gpack[:, s, :], src)
                else:
                    nc.vector.tensor_copy(gpack[:, s, :], src)

            # copy pair-major so each PSUM bank is released after ~one copy;
            # flush to HBM incrementally so the final flush is small
            for j in range(8):
                for h in range(2):
                    copy_out(psA[j][:, h, :])
                if j == 3:
                    nc.sync.dma_start(
                        out=gram_out[:, 0:8, :], in_=gpack[:, 0:8, :]
                    )
            nc.gpsimd.dma_start(out=gram_out[:, 8:16, :], in_=gpack[:, 8:16, :])

            # pass B: remaining 4 tiles (2 banks)
            psB = [
                psum.tile([P, 2, FQ], F32, tag="g", name=f"gB_{i}")
                for i in range(2)
            ]
            for r in range(NPAIR):
                for i, (mf, nq) in enumerate(PASS_B):
                    nc.tensor.matmul(
                        psB[i % 2][:, i // 2, :],
                        pair(r)[:, :, mf * P : (mf + 1) * P],
                        pair(r)[:, :, nq * FQ : (nq + 1) * FQ],
                        start=(r == 0 and i < 2),
                        stop=(r == NPAIR - 1),
                        perf_mode=DR,
                    )
            for j in range(2):
                for h in range(2):
                    copy_out(psB[j][:, h, :])
            nc.sync.dma_start(out=gram_out[:, 16:20, :], in_=gpack[:, 16:20, :])

    nc.compile()
    return nc


def build_phase2() -> bass.Bass:
    """Per-core, transposed orientation:
        Y0^T[f, n] = X^T[f, n] + sum_k E[k, f] * X^T[k, n]
    X^T ships ONCE as bf16; the fp8 copy for the DoubleRow matmuls is
    converted on-chip by the otherwise-idle Pool/Act engines. E (x64 fp8)
    is the stationary operand; the bf16 X^T is the identity-add input of
    the fused DVE scalar_tensor_tensor. Output is Y^T in bf16 (the host
    transposes back and adds b)."""
    nc = bacc.Bacc(None, target_bir_lowering=False, debug=False)

    xt_in = nc.dram_tensor("xt", [F, NC_ROWS], BF16, kind="ExternalInput")
    e_in = nc.dram_tensor("e", [F, F], F8, kind="ExternalInput")
    y_out = nc.dram_tensor("y", [F, NC_ROWS], BF16, kind="ExternalOutput")

    xt_r = xt_in.rearrange("(kb p) n -> p kb n", p=P)  # [128, 8, NC_ROWS]
    e_r = e_in.rearrange("(kb p) f -> p kb f", p=P)    # [128, 8, F]
    y_r = y_out.rearrange("(kb p) n -> p kb n", p=P)   # [128, 8, NC_ROWS]

    NCH = NC_ROWS // FH  # 8 sample chunks of 512

    with tile.TileContext(nc) as tc:
        with (
            tc.tile_pool(name="singles", bufs=1) as singles,
            tc.tile_pool(name="yout", bufs=6) as yout,
            tc.tile_pool(name="psum", bufs=8, space="PSUM") as psum,
        ):
            x16t = singles.tile([P, KB, NC_ROWS], BF16)
            x8t = singles.tile([P, KB, NC_ROWS], F8)
            e8 = singles.tile([P, KB, F], F8)
            # e8 first (every matmul's stationary), then X^T in n-chunks
            # alternating sync/scalar; each chunk is converted to fp8 by
            # Pool/Act as soon as it lands
            # sync owns ALL input DMA issue; Pool/Act are pure converters
            # at 256-sample granularity so the first fp8 data lands early
            # k-pair 0 of E unblocks every chunk's first matmul; land it first
            nc.gpsimd.dma_start(out=e8[:, 0:2, :], in_=e_r[:, 0:2, :])
            nc.gpsimd.dma_start(out=e8[:, 2:4, :], in_=e_r[:, 2:4, :])
            nc.gpsimd.dma_start(out=e8[:, 4:8, :], in_=e_r[:, 4:8, :])
            # all casts on Act: Pool's CAST runs ~3.5x slower (7us vs 2us
            # per 256-sample chunk) and would gate the PE
            for c in range(2 * NCH):
                sl = slice(c * FQ, (c + 1) * FQ)
                nc.sync.dma_start(out=x16t[:, :, sl], in_=xt_r[:, :, sl])
                nc.scalar.copy(x8t[:, :, sl], x16t[:, :, sl])

            for c in range(NCH):
                sl = slice(c * FH, (c + 1) * FH)
                ysb = yout.tile([P, KB, FH], BF16, tag="y", name=f"y_{c}")
                for fb in range(KB):
                    npairs = fb // 2 + 1  # E[k,f]=0 for k>f
                    psy = psum.tile(
                        [P, FH], F32, tag="psy", name=f"psy_{c}_{fb}"
                    )
                    for kp in range(npairs):
                        nc.tensor.matmul(
                            psy,
                            e8[:, 2 * kp : 2 * kp + 2, fb * P : (fb + 1) * P],
                            x8t[:, 2 * kp : 2 * kp + 2, sl],
                            start=(kp == 0),
                            stop=(kp == npairs - 1),
                            perf_mode=DR,
                        )
                    # yT = psum/E_SCALE + X^T (identity term, bf16-exact)
                    nc.vector.scalar_tensor_tensor(
                        ysb[:, fb, :],
                        psy,
                        1.0 / E_SCALE,
                        x16t[:, fb, sl],
                        op0=ALU.mult,
                        op1=ALU.add,
                    )
                nc.sync.dma_start(out=y_r[:, :, sl], in_=ysb)

    nc.compile()
    return nc


_programs: dict = {}


def _get_programs():
    if "p1" not in _programs:
        _programs["p1"] = build_phase1()
        _programs["p2"] = build_phase2()
    return _programs["p1"], _programs["p2"]


def kernel(X, running_mean, running_cov, beta, trace=False):
    X = np.ascontiguousarray(np.asarray(X, dtype=np.float32))
    beta = np.asarray(beta, dtype=np.float32)
    assert X.shape == (N_TOTAL, F)

    p1, p2 = _get_programs()
    core_ids = list(range(N_CORES))
    shards = X.reshape(N_CORES, NC_ROWS, F)

    tkw = {"trace_cores": core_ids} if trace else {}

    def _run(prog, in_maps):
        try:
            return run_bass_kernel_spmd(prog, in_maps, core_ids, trace=trace, **tkw)
        except Exception:
            # transient NRT/device hiccups have been observed; retry once
            import time as _time

            _time.sleep(2.0)
            return run_bass_kernel_spmd(prog, in_maps, core_ids, trace=trace, **tkw)

    shards8 = shards.astype(NP_F8)
    in1 = [{"x": shards8[i]} for i in range(N_CORES)]
    r1 = _run(p1, in1)
    kernel.exec_ns_phase1 = r1.exec_time_ns

    # unpack [128, 20, 256] tiles (in COPY_ORDER) into the dense gram
    gram = np.zeros((F, F), dtype=np.float64)
    for res in r1.results:
        g = res["gram"].astype(np.float64)
        for s, (mf, nq) in enumerate(COPY_ORDER):
            gram[mf * P : (mf + 1) * P, nq * FQ : (nq + 1) * FQ] += g[:, s, :]
    # mirror the computed lower triangle onto the upper
    gram = np.tril(gram) + np.tril(gram, -1).T

    # mean on host from the fp8-quantized X (same data the gram saw)
    colsum = shards8.astype(np.float32).sum(axis=(0, 1), dtype=np.float64)
    mean = colsum / N_TOTAL
    cov = gram / N_TOTAL - np.outer(mean, mean)
    a = cov + EPS * np.eye(F, dtype=np.float64)
    L = np.linalg.cholesky(a)
    w = np.linalg.solve(L, np.eye(F, dtype=np.float64))  # W = L^-1
    wt = np.triu(w.T)
    e8 = np.ascontiguousarray((wt - np.eye(F)) * E_SCALE).astype(NP_F8)
    b = (beta.astype(np.float64) - w @ mean).astype(np.float32)

    xt16 = np.ascontiguousarray(shards.transpose(0, 2, 1)).astype(NP_BF16)
    in2 = [{"xt": xt16[i], "e": e8} for i in range(N_CORES)]
    r2 = _run(p2, in2)
    kernel.exec_ns_phase2 = r2.exec_time_ns

    # device returns Y0^T = X^T + (X@E)^T in bf16; transpose back and add
    # the small bias here, exact
    y = np.concatenate(
        [res["y"].astype(np.float32).T for res in r2.results], axis=0
    )
    y += b[None, :]
    return y


kernel.exec_ns_phase1 = None
kernel.exec_ns_phase2 = None


# revision 39
# speedup vs baseline: 1.0410x; 1.0410x over previous
"""Batch whitening (Cholesky) kernel for Trainium2, 8 NeuronCores.

Computes, for X [32768, 1024] (matching the reference nn_BWCholeskyBlock):
    mean = X.mean(0); xc = X - mean; cov = xc.T @ xc / N
    L = chol(cov + eps I);  Y = (L^-1 xc^T).T + beta

Strategy (data-parallel over batch, 8 cores):
  Phase 1 (device): per-core partial gram  G_i = X_i^T X_i  in fp8-e4m3
     with DoubleRow matmuls (2 k-tiles contracted per instruction = 2x PE
     throughput; only the 20 lower-triangle [128,256] tiles computed).
     Tiles are packed into one SBUF buffer and shipped with 2 large DMAs
     (SW-DGE descriptor reclaim at drain costs ~130ns per DMA issued, so
     few/large transfers beat many/small ones).
  Host: reduce partials (the all-reduce of the sharding hint), mirror the
     triangle; mean from a host column sum; cov; Cholesky + triangular
     inverse of the small [F,F] factor (replicated);  W = L^-1.
  Phase 2 (device): with  WT = W.T = I + E  (E upper triangular, small),
     Y = X @ WT + b = (X + b) + X @ E.   X+b ships as bf16 (the exact
     identity part), E ships as fp8 scaled by 64, X^T ships as fp8 for
     the stationary operand.  PE does only the 6 DoubleRow E-matmuls per
     row-tile; one DVE scalar_tensor_tensor per half fuses the 1/64
     rescale with the identity add; outputs are written per row-tile PAIR
     ([128,2,1024] = 1MB per DMA) alternating across two engine queues.
"""
import sys

sys.path.insert(0, "/opt/trn_rl_repo")

import numpy as np
import ml_dtypes

import concourse.bass as bass
import concourse.mybir as mybir
import concourse.tile as tile
from concourse import bacc
from concourse.bass_utils import run_bass_kernel_spmd

EPS = 1e-5
N_CORES = 8
N_TOTAL = 32768
F = 1024
NC_ROWS = N_TOTAL // N_CORES  # 4096 rows per core
NT = NC_ROWS // 128           # 32 row-tiles per core
NPAIR = NT // 2               # 16 DoubleRow pairs per core
P = 128
FH = F // 2                   # 512
FQ = F // 4                   # 256
KB = F // P                   # 8 column blocks of 128

F32 = mybir.dt.float32
BF16 = mybir.dt.bfloat16
F8 = mybir.dt.float8e4
DR = mybir.MatmulPerfMode.DoubleRow
ALU = mybir.AluOpType

NP_F8 = ml_dtypes.float8_e4m3
NP_BF16 = ml_dtypes.bfloat16

E_SCALE = 64.0  # E entries ~5e-3 sit in fp8 subnormal range; prescale

# gram tiles (mf, nq): rows mf*128..+128, cols nq*256..+256; the 20 tiles
# covering the diagonal/lower triangle, emitted grouped by stationary mf so
# consecutive matmuls share the loaded weights. Pass A = 16 tiles (8 PSUM
# banks, 2 half-bank accumulators each), pass B = 4 tiles.
PASS_A = [
    (0, 0), (1, 0), (2, 0), (2, 1), (3, 0), (3, 1), (4, 0), (4, 1),
    (4, 2), (5, 0), (5, 1), (5, 2), (6, 0), (6, 1), (7, 0), (7, 1),
]
PASS_B = [(6, 2), (6, 3), (7, 2), (7, 3)]
# PSUM->SBUF copy order: pass A pair-major (j, h), then pass B
COPY_ORDER = [PASS_A[j + h * 8] for j in range(8) for h in range(2)] + [
    PASS_B[j + h * 2] for j in range(2) for h in range(2)
]


def build_phase1() -> bass.Bass:
    """Per-core: lower-triangle gram tiles of X^T X (fp8 DoubleRow),
    packed as [128, 20, 256] in COPY_ORDER."""
    nc = bacc.Bacc(None, target_bir_lowering=False, debug=False)

    x_in = nc.dram_tensor("x", [NC_ROWS, F], F8, kind="ExternalInput")
    gram_out = nc.dram_tensor("gram", [P, 20, FQ], F32, kind="ExternalOutput")

    x_r = x_in.rearrange("(t p) f -> p t f", p=P)  # [128, 32, 1024]

    with tile.TileContext(nc) as tc:
        with (
            tc.tile_pool(name="xres", bufs=1) as xres,
            tc.tile_pool(name="gout", bufs=1) as gout,
            tc.tile_pool(name="psum", bufs=8, space="PSUM") as psum,
        ):
            # all of X SBUF-resident in fp8 (32 KiB per partition); few
            # large DMAs (first two small so the PE starts early), split
            # across two queues
            x8 = xres.tile([P, NT, F], F8)
            # scalar's queue boots slowest (ACT table load): give it only
            # late-needed batches
            nc.sync.dma_start(out=x8[:, 0:2, :], in_=x_r[:, 0:2, :])
            nc.gpsimd.dma_start(out=x8[:, 2:6, :], in_=x_r[:, 2:6, :])
            nc.sync.dma_start(out=x8[:, 6:14, :], in_=x_r[:, 6:14, :])
            nc.scalar.dma_start(out=x8[:, 14:22, :], in_=x_r[:, 14:22, :])
            nc.gpsimd.dma_start(out=x8[:, 22:28, :], in_=x_r[:, 22:28, :])
            nc.scalar.dma_start(out=x8[:, 28:32, :], in_=x_r[:, 28:32, :])

            gpack = gout.tile([P, 20, FQ], F32)

            def pair(r):
                return x8[:, 2 * r : 2 * r + 2, :]

            # pass A: 16 gram tiles; two [128,256] fp32 accumulators share
            # one PSUM bank (bank b holds tiles idx b and b+8). start=True
            # zeroes the whole 2KB bank, so only idx<8 carries it; idx>=8
            # first matmuls land on the already-zeroed half.
            psA = [
                psum.tile([P, 2, FQ], F32, tag="g", name=f"gA_{i}")
                for i in range(8)
            ]
            for r in range(NPAIR):
                for i, (mf, nq) in enumerate(PASS_A):
                    nc.tensor.matmul(
                        psA[i % 8][:, i // 8, :],
                        pair(r)[:, :, mf * P : (mf + 1) * P],
                        pair(r)[:, :, nq * FQ : (nq + 1) * FQ],
                        start=(r == 0 and i < 8),
                        stop=(r == NPAIR - 1),
                        perf_mode=DR,
                    )

            slot = [0]

            def copy_out(src):
                s = slot[0]
                slot[0] = s + 1
                # GpSimd cannot read PSUM; alternate the PSUM-capable engines
                if s % 2 == 0:
                    nc.scalar.copy(gpack[:, s, :], src)
                else:
                    nc.vector.tensor_copy(gpack[:, s, :], src)

            # copy pair-major so each PSUM bank is released after ~one copy;
            # flush to HBM incrementally so the final flush is small
            for j in range(8):
                for h in range(2):
                    copy_out(psA[j][:, h, :])
                if j == 3:
                    nc.sync.dma_start(
                        out=gram_out[:, 0:8, :], in_=gpack[:, 0:8, :]
                    )
            nc.gpsimd.dma_start(out=gram_out[:, 8:16, :], in_=gpack[:, 8:16, :])

            # pass B: remaining 4 tiles (2 banks)
            psB = [
                psum.tile([P, 2, FQ], F32, tag="g", name=f"gB_{i}")
                for i in range(2)
            ]
            for r in range(NPAIR):
                for i, (mf, nq) in enumerate(PASS_B):
                    nc.tensor.matmul(
                        psB[i % 2][:, i // 2, :],
                        pair(r)[:, :, mf * P : (mf + 1) * P],
                        pair(r)[:, :, nq * FQ : (nq + 1) * FQ],
                        start=(r == 0 and i < 2),
                        stop=(r == NPAIR - 1),
                        perf_mode=DR,
                    )
            for j in range(2):
                for h in range(2):
                    copy_out(psB[j][:, h, :])
            nc.sync.dma_start(out=gram_out[:, 16:20, :], in_=gpack[:, 16:20, :])

    nc.compile()
    return nc


def build_phase2() -> bass.Bass:
    """Per-core, transposed orientation:
        Y0^T[f, n] = X^T[f, n] + sum_k E[k, f] * X^T[k, n]
    X^T ships ONCE as bf16; the fp8 copy for the DoubleRow matmuls is
    converted on-chip by the otherwise-idle Pool/Act engines. E (x64 fp8)
    is the stationary operand; the bf16 X^T is the identity-add input of
    the fused DVE scalar_tensor_tensor. Output is Y^T in bf16 (the host
    transposes back and adds b)."""
    nc = bacc.Bacc(None, target_bir_lowering=False, debug=False)

    xt_in = nc.dram_tensor("xt", [F, NC_ROWS], BF16, kind="ExternalInput")
    e_in = nc.dram_tensor("e", [F, F], F8, kind="ExternalInput")
    y_out = nc.dram_tensor("y", [F, NC_ROWS], BF16, kind="ExternalOutput")

    xt_r = xt_in.rearrange("(kb p) n -> p kb n", p=P)  # [128, 8, NC_ROWS]
    e_r = e_in.rearrange("(kb p) f -> p kb f", p=P)    # [128, 8, F]
    y_r = y_out.rearrange("(kb p) n -> p kb n", p=P)   # [128, 8, NC_ROWS]

    NCH = NC_ROWS // FH  # 8 sample chunks of 512

    with tile.TileContext(nc) as tc:
        with (
            tc.tile_pool(name="singles", bufs=1) as singles,
            tc.tile_pool(name="yout", bufs=6) as yout,
            tc.tile_pool(name="psum", bufs=8, space="PSUM") as psum,
        ):
            x16t = singles.tile([P, KB, NC_ROWS], BF16)
            x8t = singles.tile([P, KB, NC_ROWS], F8)
            e8 = singles.tile([P, KB, F], F8)
            # e8 first (every matmul's stationary), then X^T in n-chunks
            # alternating sync/scalar; each chunk is converted to fp8 by
            # Pool/Act as soon as it lands
            # sync owns ALL input DMA issue; Pool/Act are pure converters
            # at 256-sample granularity so the first fp8 data lands early
            # k-pair 0 of E unblocks every chunk's first matmul; land it first
            nc.gpsimd.dma_start(out=e8[:, 0:2, :], in_=e_r[:, 0:2, :])
            nc.gpsimd.dma_start(out=e8[:, 2:4, :], in_=e_r[:, 2:4, :])
            nc.gpsimd.dma_start(out=e8[:, 4:8, :], in_=e_r[:, 4:8, :])
            # all casts on Act: Pool's CAST runs ~3.5x slower (7us vs 2us
            # per 256-sample chunk) and would gate the PE
            for c in range(2 * NCH):
                sl = slice(c * FQ, (c + 1) * FQ)
                nc.sync.dma_start(out=x16t[:, :, sl], in_=xt_r[:, :, sl])
                nc.scalar.copy(x8t[:, :, sl], x16t[:, :, sl])

            for c in range(NCH):
                sl = slice(c * FH, (c + 1) * FH)
                ysb = yout.tile([P, KB, FH], BF16, tag="y", name=f"y_{c}")
                for fb in range(KB):
                    npairs = fb // 2 + 1  # E[k,f]=0 for k>f
                    psy = psum.tile(
                        [P, FH], F32, tag="psy", name=f"psy_{c}_{fb}"
                    )
                    for kp in range(npairs):
                        nc.tensor.matmul(
                            psy,
                            e8[:, 2 * kp : 2 * kp + 2, fb * P : (fb + 1) * P],
                            x8t[:, 2 * kp : 2 * kp + 2, sl],
                            start=(kp == 0),
                            stop=(kp == npairs - 1),
                            perf_mode=DR,
                        )
                    # yT = psum/E_SCALE + X^T (identity term, bf16-exact)
                    nc.vector.scalar_tensor_tensor(
                        ysb[:, fb, :],
                        psy,
                        1.0 / E_SCALE,
                        x16t[:, fb, sl],
                        op0=ALU.mult,
                        op1=ALU.add,
                    )
                    if c == NCH - 1:
                        # final chunk drains per-fb so the tail wire
                        # overlaps the remaining stts
                        nc.sync.dma_start(
                            out=y_r[:, fb, sl], in_=ysb[:, fb, :]
                        )
                if c < NCH - 1:
                    nc.sync.dma_start(out=y_r[:, :, sl], in_=ysb)

    nc.compile()
    return nc


_programs: dict = {}


def _get_programs():
    if "p1" not in _programs:
        _programs["p1"] = build_phase1()
        _programs["p2"] = build_phase2()
    return _programs["p1"], _programs["p2"]


def kernel(X, running_mean, running_cov, beta, trace=False):
    X = np.ascontiguousarray(np.asarray(X, dtype=np.float32))
    beta = np.asarray(beta, dtype=np.float32)
    assert X.shape == (N_TOTAL, F)

    p1, p2 = _get_programs()
    core_ids = list(range(N_CORES))
    shards = X.reshape(N_CORES, NC_ROWS, F)

    tkw = {"trace_cores": core_ids} if trace else {}

    def _run(prog, in_maps):
        try:
            return run_bass_kernel_spmd(prog, in_maps, core_ids, trace=trace, **tkw)
        except Exception:
            # transient NRT/device hiccups have been observed; retry once
            import time as _time

            _time.sleep(2.0)
            return run_bass_kernel_spmd(prog, in_maps, core_ids, trace=trace, **tkw)

    shards8 = shards.astype(NP_F8)
    in1 = [{"x": shards8[i]} for i in range(N_CORES)]
    r1 = _run(p1, in1)
    kernel.exec_ns_phase1 = r1.exec_time_ns

    # unpack [128, 20, 256] tiles (in COPY_ORDER) into the dense gram
    gram = np.zeros((F, F), dtype=np.float64)
    for res in r1.results:
        g = res["gram"].astype(np.float64)
        for s, (mf, nq) in enumerate(COPY_ORDER):
            gram[mf * P : (mf + 1) * P, nq * FQ : (nq + 1) * FQ] += g[:, s, :]
    # mirror the computed lower triangle onto the upper
    gram = np.tril(gram) + np.tril(gram, -1).T

    # mean on host from the fp8-quantized X (same data the gram saw)
    colsum = shards8.astype(np.float32).sum(axis=(0, 1), dtype=np.float64)
    mean = colsum / N_TOTAL
    cov = gram / N_TOTAL - np.outer(mean, mean)
    a = cov + EPS * np.eye(F, dtype=np.float64)
    L = np.linalg.cholesky(a)
    w = np.linalg.solve(L, np.eye(F, dtype=np.float64))  # W = L^-1
    wt = np.triu(w.T)
    e8 = np.ascontiguousarray((wt - np.eye(F)) * E_SCALE).astype(NP_F8)
    b = (beta.astype(np.float64) - w @ mean).astype(np.float32)

    xt16 = np.ascontiguousarray(shards.transpose(0, 2, 1)).astype(NP_BF16)
    in2 = [{"xt": xt16[i], "e": e8} for i in range(N_CORES)]
    r2 = _run(p2, in2)
    kernel.exec_ns_phase2 = r2.exec_time_ns

    # device returns Y0^T = X^T + (X@E)^T in bf16; transpose back and add
    # the small bias here, exact
    y = np.concatenate(
        [res["y"].astype(np.float32).T for res in r2.results], axis=0
    )
    y += b[None, :]
    return y


kernel.exec_ns_phase1 = None
kernel.exec_ns_phase2 = None
